# revision 6
# baseline (speedup 1.0000x reference)
"""Trainium2 Bass kernel for DynamicPTTopicModeling.

Computes, per batch b (one batch per NeuronCore, 8 cores):
    qg  = relu(qz @ bw.T)            # [R=8192, G=512], contraction over d=1024
    den = max(sum_g qg, 1e-6)        # per-row L1 norm
    msg = (qg @ bw) / den            # [R, D=1024]

Sharding: batch b across the 8 NeuronCores, fully data-parallel.

All matmuls run in bf16 (measured on hw: 216ns per 512-free matmul, same
stream rate as f32r but ldweights is 97ns vs 181ns and all DMA traffic is
halved; end-to-end rel err ~3.5e-3, well inside the 2e-2 gate). fp8
DoubleRow was measured at the same 216ns/instr (2x MACs/instr), but the
hi/lo 3-term split needed to hit the accuracy gate costs 1.5x MACs versus
bf16's 1.0x, so bf16 wins.

The PE contracts over the partition dim for both operands, so qz enters
mm1 with d on partitions: host pre-transposes qz/bw while building the
per-core shards (layout marshalling, free wrt HW time).

Per-core strategy (16 mega-tiles of 512 rows):
  - mm1 produces qg TRANSPOSED ([g, p] layout): stationary = bwT slices,
    moving = qzT chunks (N=512). Relu'd into bf16 qgr, which mm2 then
    consumes directly as its stationary.
  - The per-row L1 denominator comes FOR FREE out of mm2: the mm2 moving
    operand is bw augmented with a leading all-ones column ([G, 1+D]), so
    psum column 0 of the first free-group accumulates sum_g qg[g, p] in
    f32 exactly for the same qgr the products use. No separate row-sum
    matmuls, no PE transposes. Since a psum accumulation group caps the
    free dim at 512 f32, the 1025 output columns split into free-groups
    of 343/342/340.
  - max(den, eps) + reciprocal run per-partition on [128, 1]; the scale is
    applied on the psum->SBUF copies (vector/scalar engines alternate),
    which write bf16 msg tiles that DMA out on the gpsimd ring.
"""
from contextlib import ExitStack

import numpy as np
import ml_dtypes

import concourse.bass as bass
import concourse.tile as tile
from concourse import bacc, mybir
from concourse.bass_utils import run_bass_kernel_spmd

F32 = mybir.dt.float32
BF16 = mybir.dt.bfloat16
AF = mybir.ActivationFunctionType
NP_BF16 = ml_dtypes.bfloat16

B, C, P, D, G = 8, 16, 512, 1024, 512
R = C * P            # 8192 rows per batch
MEGA = 512           # rows per mega-tile
NSUB = MEGA // 128   # 4
NMEGA = R // MEGA    # 16
KD = D // 128        # 8 d-chunks
KG = G // 128        # 4 g-chunks
DA = D + 1           # mm2 moving columns: [ones | bw]
# mm2 free-groups (psum bank caps a group at 512 f32 columns)
MM2_GROUPS = [(0, 343), (343, 685), (685, 1025)]
EPS = 1e-6
N_CORES = 8


def build_kernel():
    nc = bacc.Bacc("TRN2", target_bir_lowering=False)
    qzT_d = nc.dram_tensor("qzT", [D, R], BF16, kind="ExternalInput")
    bwT_d = nc.dram_tensor("bwT", [D, G], BF16, kind="ExternalInput")
    bwA_d = nc.dram_tensor("bwA", [G, DA], BF16, kind="ExternalInput")
    msg_d = nc.dram_tensor("msg", [R, D], BF16, kind="ExternalOutput")

    with tile.TileContext(nc) as tc, ExitStack() as ctx:
        const_pool = ctx.enter_context(tc.tile_pool(name="const", bufs=1))
        in_pool = ctx.enter_context(tc.tile_pool(name="inp", bufs=2))
        qgr_pool = ctx.enter_context(tc.tile_pool(name="qgrp", bufs=2))
        out_pool = ctx.enter_context(tc.tile_pool(name="outp", bufs=2))
        small_pool = ctx.enter_context(tc.tile_pool(name="smallp", bufs=4))
        qg_psum = ctx.enter_context(tc.tile_pool(name="qgps", bufs=2, space="PSUM"))
        msg_psum = ctx.enter_context(tc.tile_pool(name="msgps", bufs=5, space="PSUM"))

        # Weights go on the scalar-engine DGE ring so they don't queue behind
        # the qzT stream. One DMA moves ~55GB/s on a single engine, so bwT
        # goes per-k-chunk (128KB each): the first mm1 ldweights only needs
        # chunk k=0, and the 8 chunks overlap in flight.
        bwT_sb = const_pool.tile([128, KD, G], BF16)
        bwT_view = bwT_d[:].rearrange("(k p) g -> p k g", p=128)
        for k in range(KD):
            nc.scalar.dma_start(
                out=bwT_sb[:, k, :], in_=bwT_view[:, k, :]
            )
        # bw augmented [g, 1+d] -> [128, gc, DA]  (mm2 moving operand)
        bw_sb = const_pool.tile([128, KG, DA], BF16)
        nc.scalar.dma_start(
            out=bw_sb, in_=bwA_d[:].rearrange("(gc p) d -> p gc d", p=128)
        )

        def load_qzT(t):
            # The first two megas gate the pipeline start: split them into
            # per-k slices spread over the sync AND vector rings so up to 8
            # transfers run on distinct DMA engines. Steady state uses halves
            # on the sync ring (fits well inside the mega period).
            qzT = in_pool.tile([128, KD, MEGA], BF16, name="qzT")
            qzT_view = qzT_d[:, t * MEGA:(t + 1) * MEGA].rearrange(
                "(k p) r -> p k r", p=128
            )
            if t < 2:
                for k in range(KD):
                    eng = nc.sync if k % 2 == 0 else nc.gpsimd
                    eng.dma_start(
                        out=qzT[:, k, :], in_=qzT_view[:, k, :]
                    )
            else:
                for q in range(2):
                    nc.sync.dma_start(
                        out=qzT[:, 4 * q:4 * (q + 1), :],
                        in_=qzT_view[:, 4 * q:4 * (q + 1), :],
                    )
            return qzT

        pend_qzT = [load_qzT(0), load_qzT(1)]

        for t in range(NMEGA):
            qzT = pend_qzT.pop(0)
            if t + 2 < NMEGA:
                pend_qzT.append(load_qzT(t + 2))

            # ---- mm1: qgT[gc] = sum_k bwT[:,k,gc].T @ qzT[:,k,:] -> relu ----
            qgr = qgr_pool.tile([128, KG, MEGA], BF16, name="qgr")
            for gc in range(KG):
                qg_ps = qg_psum.tile([128, MEGA], F32, name="qg_ps")
                for k in range(KD):
                    nc.tensor.matmul(
                        qg_ps,
                        bwT_sb[:, k, gc * 128:(gc + 1) * 128],
                        qzT[:, k, :],
                        start=(k == 0),
                        stop=(k == KD - 1),
                    )
                nc.scalar.activation(qgr[:, gc, :], qg_ps, AF.Relu)

            # ---- mm2: msg[s] = sum_gc qgr[:,gc,s].T @ bwA[gc], scaled by
            # 1/max(psum col 0, eps) which holds sum_g qgr for these rows ----
            msg_sb = out_pool.tile([128, NSUB, D], BF16, name="msg_sb")
            for s in range(NSUB):
                sc_sb = None
                for gi, (c0, c1) in enumerate(MM2_GROUPS):
                    m_ps = msg_psum.tile([128, c1 - c0], F32, name="m_ps")
                    for gc in range(KG):
                        nc.tensor.matmul(
                            m_ps,
                            qgr[:, gc, s * 128:(s + 1) * 128],
                            bw_sb[:, gc, c0:c1],
                            start=(gc == 0),
                            stop=(gc == KG - 1),
                        )
                    if gi == 0:
                        sc_sb = small_pool.tile([128, 1], F32, name="sc_sb")
                        nc.vector.tensor_scalar_max(sc_sb, m_ps[:, 0:1], EPS)
                        nc.vector.reciprocal(sc_sb, sc_sb)
                        nc.vector.tensor_scalar_mul(
                            msg_sb[:, s, 0:342], m_ps[:, 1:343], sc_sb
                        )
                    elif gi == 1:
                        nc.scalar.mul(
                            msg_sb[:, s, c0 - 1:c1 - 1], m_ps, sc_sb
                        )
                    else:
                        nc.vector.tensor_scalar_mul(
                            msg_sb[:, s, c0 - 1:c1 - 1], m_ps, sc_sb
                        )
                # per-sub store as two 128KB halves on the gpsimd + scalar
                # rings: two DMA engines drain them in parallel (~2.3us each)
                # so the final sub's store doesn't lengthen the tail, and
                # neither ring queues behind the qzT loads (sync ring). The
                # scalar ring is idle once the weights are in.
                row0 = t * MEGA + s * 128
                nc.gpsimd.dma_start(
                    out=msg_d[row0:row0 + 128, 0:512],
                    in_=msg_sb[:, s, 0:512],
                )
                nc.scalar.dma_start(
                    out=msg_d[row0:row0 + 128, 512:1024],
                    in_=msg_sb[:, s, 512:1024],
                )

    nc.compile()
    return nc


_NC_CACHE = None


def _get_nc():
    global _NC_CACHE
    if _NC_CACHE is None:
        _NC_CACHE = build_kernel()
    return _NC_CACHE


def kernel(qz: np.ndarray, binary_weight: np.ndarray) -> np.ndarray:
    qz = np.asarray(qz, dtype=np.float32)
    bw = np.asarray(binary_weight, dtype=np.float32)
    assert qz.shape == (B, C, P, D), qz.shape
    assert bw.shape == (B, G, D), bw.shape

    nc = _get_nc()
    ones = np.ones((G, 1), dtype=np.float32)
    in_maps = []
    for i in range(N_CORES):
        qzT = np.ascontiguousarray(qz[i].reshape(R, D).T).astype(NP_BF16)
        bwT = np.ascontiguousarray(bw[i].T).astype(NP_BF16)
        bwA = np.concatenate([ones, bw[i]], axis=1).astype(NP_BF16)
        in_maps.append({"qzT": qzT, "bwT": bwT, "bwA": bwA})
    res = run_bass_kernel_spmd(nc, in_maps, core_ids=list(range(N_CORES)))
    out = np.stack(
        [
            res.results[i]["msg"].astype(np.float32).reshape(C, P, D)
            for i in range(N_CORES)
        ],
        axis=0,
    )
    return out


# revision 10
# speedup vs baseline: 1.0092x; 1.0092x over previous
"""Trainium2 Bass kernel for DynamicPTTopicModeling.

Computes, per batch b (one batch per NeuronCore, 8 cores):
    qg  = relu(qz @ bw.T)            # [R=8192, G=512], contraction over d=1024
    den = max(sum_g qg, 1e-6)        # per-row L1 norm
    msg = (qg @ bw) / den            # [R, D=1024]

Sharding: batch b across the 8 NeuronCores, fully data-parallel.

All matmuls run in bf16 (measured on hw: ~216ns per 512-free matmul, the
same stream rate as f32r, but ldweights drops 181->97ns and all DMA
traffic is halved; end-to-end rel err ~3.2e-3 vs the 2e-2 gate). fp8
DoubleRow was measured at the same 216ns/instr (so 2x MACs/instr), but
the hi/lo 3-term split needed for the accuracy gate costs 1.5x MACs
versus bf16's 1.0x, so bf16 wins.

The PE contracts over the partition dim for both operands, so qz enters
mm1 with d on partitions: host pre-transposes qz/bw while building the
per-core shards (layout marshalling, free wrt HW time).

Row tiles ("megas") are graduated 128/384/512...: a small first tile
needs only 256KB of qzT before mm1 can run its full k-chain, so the PE
starts ~2us earlier and never underruns while the DMA rings ramp
(a single DMA moves ~55GB/s; a ring overlaps ~4 in flight). A burst of
dummy matmuls on a zeroed scratch tile runs during the DMA head to walk
the PE out of its cold p-state before the real work lands.

Per mega:
  - mm1 produces qg TRANSPOSED ([g, p] layout): stationary = bwT slices,
    moving = qzT chunks. Relu'd into bf16 qgr, which mm2 then consumes
    directly as its stationary.
  - The per-row L1 denominator comes FOR FREE out of mm2: the mm2 moving
    operand is bw augmented with a leading all-ones column ([G, 1+D]), so
    psum column 0 of the first free-group accumulates sum_g qg[g, p] in
    f32 for exactly the qgr the products use. No separate row-sum
    matmuls, no PE transposes. Since a psum accumulation group caps the
    free dim at 512 f32, the 1025 output columns split into free-groups
    of 343/342/340.
  - max(den, eps) + reciprocal run per-partition on [128, 1]; the scale
    is applied on the psum->SBUF copies (vector/scalar engines), which
    write bf16 msg tiles stored as two 128KB halves on the gpsimd +
    scalar DMA rings (the sync ring carries the qzT loads; the last mega
    rotates over all three rings to shorten the final drain).
"""
from contextlib import ExitStack

import numpy as np
import ml_dtypes

import concourse.bass as bass
import concourse.tile as tile
from concourse import bacc, mybir
from concourse.bass_utils import run_bass_kernel_spmd

F32 = mybir.dt.float32
BF16 = mybir.dt.bfloat16
AF = mybir.ActivationFunctionType
NP_BF16 = ml_dtypes.bfloat16

B, C, P, D, G = 8, 16, 512, 1024, 512
R = C * P            # 8192 rows per batch
KD = D // 128        # 8 d-chunks
KG = G // 128        # 4 g-chunks
DA = D + 1           # mm2 moving columns: [ones | bw]
# mm2 free-groups (psum bank caps a group at 512 f32 columns)
MM2_GROUPS = [(0, 343), (343, 685), (685, 1025)]
EPS = 1e-6
N_CORES = 8
N_WARM = 12          # dummy matmuls to exit the PE cold p-state

# graduated row tiles: small first tiles cut the DMA-gated head
MEGA_SIZES = [128, 384] + [512] * 15
MEGA_OFFS = [sum(MEGA_SIZES[:i]) for i in range(len(MEGA_SIZES))]
assert sum(MEGA_SIZES) == R


def build_kernel():
    nc = bacc.Bacc("TRN2", target_bir_lowering=False)
    qzT_d = nc.dram_tensor("qzT", [D, R], BF16, kind="ExternalInput")
    bwT_d = nc.dram_tensor("bwT", [D, G], BF16, kind="ExternalInput")
    bwA_d = nc.dram_tensor("bwA", [G, DA], BF16, kind="ExternalInput")
    msg_d = nc.dram_tensor("msg", [R, D], BF16, kind="ExternalOutput")

    with tile.TileContext(nc) as tc, ExitStack() as ctx:
        const_pool = ctx.enter_context(tc.tile_pool(name="const", bufs=1))
        in_pool = ctx.enter_context(tc.tile_pool(name="inp", bufs=2))
        qgr_pool = ctx.enter_context(tc.tile_pool(name="qgrp", bufs=2))
        out_pool = ctx.enter_context(tc.tile_pool(name="outp", bufs=2))
        small_pool = ctx.enter_context(tc.tile_pool(name="smallp", bufs=4))
        qg_psum = ctx.enter_context(tc.tile_pool(name="qgps", bufs=2, space="PSUM"))
        msg_psum = ctx.enter_context(tc.tile_pool(name="msgps", bufs=5, space="PSUM"))
        warm_psum = ctx.enter_context(tc.tile_pool(name="warmps", bufs=1, space="PSUM"))

        # p-state warmup: the PE ramps to full clock only after ~3us of
        # continuous execution; burn the DMA-bound head on dummy matmuls.
        scratch = const_pool.tile([128, 128], BF16)
        nc.vector.memset(scratch, 0.0)
        warm_ps = warm_psum.tile([128, 128], F32)
        for _ in range(N_WARM):
            nc.tensor.matmul(warm_ps, scratch, scratch, start=True, stop=True)

        # Weights go on the scalar-engine DGE ring so they don't queue behind
        # the qzT stream; k=0 first and alone (128KB) - it gates the first
        # ldweights - then the rest in flight behind it.
        bwT_sb = const_pool.tile([128, KD, G], BF16)
        bwT_view = bwT_d[:].rearrange("(k p) g -> p k g", p=128)
        nc.scalar.dma_start(out=bwT_sb[:, 0, :], in_=bwT_view[:, 0, :])
        nc.scalar.dma_start(out=bwT_sb[:, 1, :], in_=bwT_view[:, 1, :])
        for q in range(3):
            nc.scalar.dma_start(
                out=bwT_sb[:, 2 + 2 * q:4 + 2 * q, :],
                in_=bwT_view[:, 2 + 2 * q:4 + 2 * q, :],
            )
        # bw augmented [g, 1+d] -> [128, gc, DA]  (mm2 moving operand)
        bw_sb = const_pool.tile([128, KG, DA], BF16)
        nc.scalar.dma_start(
            out=bw_sb, in_=bwA_d[:].rearrange("(gc p) d -> p gc d", p=128)
        )

        def load_qzT(idx):
            off, m = MEGA_OFFS[idx], MEGA_SIZES[idx]
            qzT = in_pool.tile([128, KD, 512], BF16, name="qzT")
            qzT_view = qzT_d[:, off:off + m].rearrange("(k p) r -> p k r", p=128)
            # two DMAs per tile (a ring overlaps them on separate engines)
            for q in range(2):
                nc.sync.dma_start(
                    out=qzT[:, 4 * q:4 * (q + 1), 0:m],
                    in_=qzT_view[:, 4 * q:4 * (q + 1), :],
                )
            return qzT

        pend_qzT = [load_qzT(0), load_qzT(1)]
        n_megas = len(MEGA_SIZES)
        store_rr = 0

        for t in range(n_megas):
            off, m = MEGA_OFFS[t], MEGA_SIZES[t]
            nsub = m // 128
            qzT = pend_qzT.pop(0)
            if t + 2 < n_megas:
                pend_qzT.append(load_qzT(t + 2))

            # ---- mm1: qgT[gc] = sum_k bwT[:,k,gc].T @ qzT[:,k,:] -> relu ----
            qgr = qgr_pool.tile([128, KG, 512], BF16, name="qgr")
            for gc in range(KG):
                qg_ps = qg_psum.tile([128, 512], F32, name="qg_ps")
                for k in range(KD):
                    nc.tensor.matmul(
                        qg_ps[:, 0:m],
                        bwT_sb[:, k, gc * 128:(gc + 1) * 128],
                        qzT[:, k, 0:m],
                        start=(k == 0),
                        stop=(k == KD - 1),
                    )
                nc.scalar.activation(qgr[:, gc, 0:m], qg_ps[:, 0:m], AF.Relu)

            # ---- mm2: msg[s] = sum_gc qgr[:,gc,s].T @ bwA[gc], scaled by
            # 1/max(psum col 0, eps) which holds sum_g qgr for these rows ----
            msg_sb = out_pool.tile([128, 4, D], BF16, name="msg_sb")
            for s in range(nsub):
                sc_sb = None
                for gi, (c0, c1) in enumerate(MM2_GROUPS):
                    m_ps = msg_psum.tile([128, c1 - c0], F32, name="m_ps")
                    for gc in range(KG):
                        nc.tensor.matmul(
                            m_ps,
                            qgr[:, gc, s * 128:(s + 1) * 128],
                            bw_sb[:, gc, c0:c1],
                            start=(gc == 0),
                            stop=(gc == KG - 1),
                        )
                    if gi == 0:
                        sc_sb = small_pool.tile([128, 1], F32, name="sc_sb")
                        nc.vector.tensor_scalar_max(sc_sb, m_ps[:, 0:1], EPS)
                        nc.vector.reciprocal(sc_sb, sc_sb)
                        nc.vector.tensor_scalar_mul(
                            msg_sb[:, s, 0:342], m_ps[:, 1:343], sc_sb
                        )
                    elif gi == 1:
                        nc.scalar.mul(
                            msg_sb[:, s, c0 - 1:c1 - 1], m_ps, sc_sb
                        )
                    else:
                        nc.vector.tensor_scalar_mul(
                            msg_sb[:, s, c0 - 1:c1 - 1], m_ps, sc_sb
                        )
                # store each sub as two 128KB halves on separate DMA rings so
                # two engines drain them in parallel; the last mega rotates
                # over all three rings (the sync ring is done loading by
                # then) so the final drain is as wide as possible
                row0 = off + s * 128
                if t == n_megas - 1:
                    rings = [nc.gpsimd, nc.scalar, nc.sync]
                    for h in range(2):
                        rings[store_rr % 3].dma_start(
                            out=msg_d[row0:row0 + 128, 512 * h:512 * (h + 1)],
                            in_=msg_sb[:, s, 512 * h:512 * (h + 1)],
                        )
                        store_rr += 1
                else:
                    nc.gpsimd.dma_start(
                        out=msg_d[row0:row0 + 128, 0:512],
                        in_=msg_sb[:, s, 0:512],
                    )
                    nc.scalar.dma_start(
                        out=msg_d[row0:row0 + 128, 512:1024],
                        in_=msg_sb[:, s, 512:1024],
                    )

    nc.compile()
    return nc


_NC_CACHE = None


def _get_nc():
    global _NC_CACHE
    if _NC_CACHE is None:
        _NC_CACHE = build_kernel()
    return _NC_CACHE


def kernel(qz: np.ndarray, binary_weight: np.ndarray) -> np.ndarray:
    qz = np.asarray(qz, dtype=np.float32)
    bw = np.asarray(binary_weight, dtype=np.float32)
    assert qz.shape == (B, C, P, D), qz.shape
    assert bw.shape == (B, G, D), bw.shape

    nc = _get_nc()
    ones = np.ones((G, 1), dtype=np.float32)
    in_maps = []
    for i in range(N_CORES):
        qzT = np.ascontiguousarray(qz[i].reshape(R, D).T).astype(NP_BF16)
        bwT = np.ascontiguousarray(bw[i].T).astype(NP_BF16)
        bwA = np.concatenate([ones, bw[i]], axis=1).astype(NP_BF16)
        in_maps.append({"qzT": qzT, "bwT": bwT, "bwA": bwA})
    res = run_bass_kernel_spmd(nc, in_maps, core_ids=list(range(N_CORES)))
    out = np.stack(
        [
            res.results[i]["msg"].astype(np.float32).reshape(C, P, D)
            for i in range(N_CORES)
        ],
        axis=0,
    )
    return out


# revision 11
# speedup vs baseline: 1.0135x; 1.0043x over previous
"""Trainium2 Bass kernel for DynamicPTTopicModeling.

Computes, per batch b (one batch per NeuronCore, 8 cores):
    qg  = relu(qz @ bw.T)            # [R=8192, G=512], contraction over d=1024
    den = max(sum_g qg, 1e-6)        # per-row L1 norm
    msg = (qg @ bw) / den            # [R, D=1024]

Sharding: batch b across the 8 NeuronCores, fully data-parallel.

All matmuls run in bf16 (measured on hw: ~216ns per 512-free matmul, the
same stream rate as f32r, but ldweights drops 181->97ns and all DMA
traffic is halved; end-to-end rel err ~3.2e-3 vs the 2e-2 gate). fp8
DoubleRow was measured at the same ~216ns/instr (2x MACs/instr), but the
hi/lo 3-term split needed for the accuracy gate costs 1.5x MACs versus
bf16's 1.0x, so bf16 wins.

The PE contracts over the partition dim for both operands, so qz enters
mm1 with d on partitions: host pre-transposes qz/bw while building the
per-core shards (layout marshalling, free wrt HW time).

The kernel head is aggregate-DMA-bandwidth-bound (~2MB of qzT+bwT must
land before the PE can run continuously; all-ring issue saturates the
~360GB/s fabric), so the schedule keeps the proven late-but-clean start:
quarter loads for the first two megas, then 1MB halves. A burst of dummy
matmuls on a zeroed scratch tile runs during that DMA head purely to
walk the PE out of its cold DVFS p-state before real work lands.

Per 512-row mega-tile (16 of them):
  - mm1 produces qg TRANSPOSED ([g, p] layout): stationary = bwT slices,
    moving = qzT chunks (N=512). Relu'd into bf16 qgr, which mm2 then
    consumes directly as its stationary.
  - The per-row L1 denominator comes FOR FREE out of mm2: the mm2 moving
    operand is bw augmented with a leading all-ones column ([G, 1+D]), so
    psum column 0 of the first free-group accumulates sum_g qg[g, p] in
    f32 for exactly the qgr the products use. No separate row-sum
    matmuls, no PE transposes. Since a psum accumulation group caps the
    free dim at 512 f32, the 1025 output columns split into free-groups
    of 343/342/340.
  - max(den, eps) + reciprocal run per-partition on [128, 1]; the scale
    is applied on the psum->SBUF copies (vector/scalar engines), which
    write bf16 msg tiles stored per-sub on the gpsimd ring (clear of the
    sync-ring qzT loads). The last mega spreads its stores as halves
    over all three rings so the final ring drain is as wide as possible.
"""
from contextlib import ExitStack

import numpy as np
import ml_dtypes

import concourse.bass as bass
import concourse.tile as tile
from concourse import bacc, mybir
from concourse.bass_utils import run_bass_kernel_spmd

F32 = mybir.dt.float32
BF16 = mybir.dt.bfloat16
AF = mybir.ActivationFunctionType
NP_BF16 = ml_dtypes.bfloat16

B, C, P, D, G = 8, 16, 512, 1024, 512
R = C * P            # 8192 rows per batch
MEGA = 512           # rows per mega-tile
NSUB = MEGA // 128   # 4
NMEGA = R // MEGA    # 16
KD = D // 128        # 8 d-chunks
KG = G // 128        # 4 g-chunks
DA = D + 1           # mm2 moving columns: [ones | bw]
# mm2 free-groups (psum bank caps a group at 512 f32 columns)
MM2_GROUPS = [(0, 343), (343, 685), (685, 1025)]
EPS = 1e-6
N_CORES = 8
N_WARM = 12          # dummy matmuls to exit the PE cold p-state


def build_kernel():
    nc = bacc.Bacc("TRN2", target_bir_lowering=False)
    qzT_d = nc.dram_tensor("qzT", [D, R], BF16, kind="ExternalInput")
    bwT_d = nc.dram_tensor("bwT", [D, G], BF16, kind="ExternalInput")
    bwA_d = nc.dram_tensor("bwA", [G, DA], BF16, kind="ExternalInput")
    msg_d = nc.dram_tensor("msg", [R, D], BF16, kind="ExternalOutput")

    with tile.TileContext(nc) as tc, ExitStack() as ctx:
        const_pool = ctx.enter_context(tc.tile_pool(name="const", bufs=1))
        in_pool = ctx.enter_context(tc.tile_pool(name="inp", bufs=2))
        qgr_pool = ctx.enter_context(tc.tile_pool(name="qgrp", bufs=2))
        out_pool = ctx.enter_context(tc.tile_pool(name="outp", bufs=2))
        small_pool = ctx.enter_context(tc.tile_pool(name="smallp", bufs=4))
        qg_psum = ctx.enter_context(tc.tile_pool(name="qgps", bufs=2, space="PSUM"))
        msg_psum = ctx.enter_context(tc.tile_pool(name="msgps", bufs=5, space="PSUM"))
        warm_psum = ctx.enter_context(tc.tile_pool(name="warmps", bufs=1, space="PSUM"))

        # p-state warmup: the PE reaches full clock only after ~3us of
        # continuous execution; burn the DMA-bound head on dummy matmuls.
        scratch = const_pool.tile([128, 128], BF16)
        nc.vector.memset(scratch, 0.0)
        warm_ps = warm_psum.tile([128, 128], F32)
        for _ in range(N_WARM):
            nc.tensor.matmul(warm_ps, scratch, scratch, start=True, stop=True)

        # Weights go on the scalar-engine DGE ring so they don't queue behind
        # the qzT stream; bwT first and in quarters - it gates mm1.
        bwT_sb = const_pool.tile([128, KD, G], BF16)
        bwT_view = bwT_d[:].rearrange("(k p) g -> p k g", p=128)
        for q in range(4):
            nc.scalar.dma_start(
                out=bwT_sb[:, 2 * q:2 * q + 2, :], in_=bwT_view[:, 2 * q:2 * q + 2, :]
            )
        # bw augmented [g, 1+d] -> [128, gc, DA]  (mm2 moving operand)
        bw_sb = const_pool.tile([128, KG, DA], BF16)
        nc.scalar.dma_start(
            out=bw_sb, in_=bwA_d[:].rearrange("(gc p) d -> p gc d", p=128)
        )

        def load_qzT(t):
            # first megas load in quarters (earlier first matmul); steady
            # state uses halves (better DMA efficiency)
            qzT = in_pool.tile([128, KD, MEGA], BF16, name="qzT")
            qzT_view = qzT_d[:, t * MEGA:(t + 1) * MEGA].rearrange(
                "(k p) r -> p k r", p=128
            )
            nq = 4 if t < 2 else 2
            step = KD // nq
            for q in range(nq):
                nc.sync.dma_start(
                    out=qzT[:, step * q:step * (q + 1), :],
                    in_=qzT_view[:, step * q:step * (q + 1), :],
                )
            return qzT

        pend_qzT = [load_qzT(0), load_qzT(1)]
        store_rr = 0

        for t in range(NMEGA):
            qzT = pend_qzT.pop(0)
            if t + 2 < NMEGA:
                pend_qzT.append(load_qzT(t + 2))

            # ---- mm1: qgT[gc] = sum_k bwT[:,k,gc].T @ qzT[:,k,:] -> relu ----
            qgr = qgr_pool.tile([128, KG, MEGA], BF16, name="qgr")
            for gc in range(KG):
                qg_ps = qg_psum.tile([128, MEGA], F32, name="qg_ps")
                for k in range(KD):
                    nc.tensor.matmul(
                        qg_ps,
                        bwT_sb[:, k, gc * 128:(gc + 1) * 128],
                        qzT[:, k, :],
                        start=(k == 0),
                        stop=(k == KD - 1),
                    )
                nc.scalar.activation(qgr[:, gc, :], qg_ps, AF.Relu)

            # ---- mm2: msg[s] = sum_gc qgr[:,gc,s].T @ bwA[gc], scaled by
            # 1/max(psum col 0, eps) which holds sum_g qgr for these rows ----
            msg_sb = out_pool.tile([128, NSUB, D], BF16, name="msg_sb")
            for s in range(NSUB):
                sc_sb = None
                for gi, (c0, c1) in enumerate(MM2_GROUPS):
                    m_ps = msg_psum.tile([128, c1 - c0], F32, name="m_ps")
                    for gc in range(KG):
                        nc.tensor.matmul(
                            m_ps,
                            qgr[:, gc, s * 128:(s + 1) * 128],
                            bw_sb[:, gc, c0:c1],
                            start=(gc == 0),
                            stop=(gc == KG - 1),
                        )
                    if gi == 0:
                        sc_sb = small_pool.tile([128, 1], F32, name="sc_sb")
                        nc.vector.tensor_scalar_max(sc_sb, m_ps[:, 0:1], EPS)
                        nc.vector.reciprocal(sc_sb, sc_sb)
                        nc.vector.tensor_scalar_mul(
                            msg_sb[:, s, 0:342], m_ps[:, 1:343], sc_sb
                        )
                    elif gi == 1:
                        nc.scalar.mul(
                            msg_sb[:, s, c0 - 1:c1 - 1], m_ps, sc_sb
                        )
                    else:
                        nc.vector.tensor_scalar_mul(
                            msg_sb[:, s, c0 - 1:c1 - 1], m_ps, sc_sb
                        )
                row0 = t * MEGA + s * 128
                if t == NMEGA - 1:
                    # spread the final stores as halves over all three DMA
                    # rings (sync is done loading): widest terminal drain
                    rings = [nc.gpsimd, nc.scalar, nc.sync]
                    for h in range(2):
                        rings[store_rr % 3].dma_start(
                            out=msg_d[row0:row0 + 128, 512 * h:512 * (h + 1)],
                            in_=msg_sb[:, s, 512 * h:512 * (h + 1)],
                        )
                        store_rr += 1
                else:
                    # per-sub store on the gpsimd ring: stays clear of the
                    # qzT loads (sync ring) and weight loads (scalar ring)
                    nc.gpsimd.dma_start(
                        out=msg_d[row0:row0 + 128, :],
                        in_=msg_sb[:, s, :],
                    )

    nc.compile()
    return nc


_NC_CACHE = None


def _get_nc():
    global _NC_CACHE
    if _NC_CACHE is None:
        _NC_CACHE = build_kernel()
    return _NC_CACHE


def kernel(qz: np.ndarray, binary_weight: np.ndarray) -> np.ndarray:
    qz = np.asarray(qz, dtype=np.float32)
    bw = np.asarray(binary_weight, dtype=np.float32)
    assert qz.shape == (B, C, P, D), qz.shape
    assert bw.shape == (B, G, D), bw.shape

    nc = _get_nc()
    ones = np.ones((G, 1), dtype=np.float32)
    in_maps = []
    for i in range(N_CORES):
        qzT = np.ascontiguousarray(qz[i].reshape(R, D).T).astype(NP_BF16)
        bwT = np.ascontiguousarray(bw[i].T).astype(NP_BF16)
        bwA = np.concatenate([ones, bw[i]], axis=1).astype(NP_BF16)
        in_maps.append({"qzT": qzT, "bwT": bwT, "bwA": bwA})
    res = run_bass_kernel_spmd(nc, in_maps, core_ids=list(range(N_CORES)))
    out = np.stack(
        [
            res.results[i]["msg"].astype(np.float32).reshape(C, P, D)
            for i in range(N_CORES)
        ],
        axis=0,
    )
    return out


# revision 14
# speedup vs baseline: 1.0230x; 1.0093x over previous
"""Trainium2 Bass kernel for DynamicPTTopicModeling.

Computes, per batch b (one batch per NeuronCore, 8 cores):
    qg  = relu(qz @ bw.T)            # [R=8192, G=512], contraction over d=1024
    den = max(sum_g qg, 1e-6)        # per-row L1 norm
    msg = (qg @ bw) / den            # [R, D=1024]

Sharding: batch b across the 8 NeuronCores, fully data-parallel.

All matmuls run in bf16 (measured on hw: ~216ns per 512-free matmul, the
same stream rate as f32r, but ldweights drops 181->97ns and all DMA
traffic is halved; end-to-end rel err ~3.2e-3 vs the 2e-2 gate). fp8
DoubleRow was measured at the same ~216ns/instr (2x MACs/instr), but the
hi/lo 3-term split needed for the accuracy gate costs 1.5x MACs versus
bf16's 1.0x, so bf16 wins.

The PE contracts over the partition dim for both operands, so qz enters
mm1 with d on partitions: host pre-transposes qz/bw while building the
per-core shards (layout marshalling, free wrt HW time).

The kernel head is aggregate-DMA-bandwidth-bound (~2MB of qzT+bwT must
land before the PE can run continuously; all-ring issue saturates the
~360GB/s fabric), so the schedule keeps the proven late-but-clean start:
quarter loads for the first two megas, then 1MB halves. A burst of dummy
matmuls on a zeroed scratch tile runs during that DMA head purely to
walk the PE out of its cold DVFS p-state before real work lands.

Per 512-row mega-tile (16 of them):
  - mm1 produces qg TRANSPOSED ([g, p] layout): stationary = bwT slices,
    moving = qzT chunks (N=512). Relu'd into bf16 qgr, which mm2 then
    consumes directly as its stationary.
  - The per-row L1 denominator comes FOR FREE out of mm2: the mm2 moving
    operand is bw augmented with a leading all-ones column ([G, 1+D]), so
    psum column 0 of the first free-group accumulates sum_g qg[g, p] in
    f32 for exactly the qgr the products use. No separate row-sum
    matmuls, no PE transposes. Since a psum accumulation group caps the
    free dim at 512 f32, the 1025 output columns split into free-groups
    of 343/342/340.
  - max(den, eps) + reciprocal run per-partition on [128, 1]; the scale
    is applied on the psum->SBUF copies (vector/scalar engines), which
    write bf16 msg tiles stored per-sub on the gpsimd ring (clear of the
    sync-ring qzT loads). The last mega spreads its stores as halves
    over all three rings so the final ring drain is as wide as possible.
"""
from contextlib import ExitStack

import numpy as np
import ml_dtypes

import concourse.bass as bass
import concourse.tile as tile
from concourse import bacc, mybir
from concourse.bass_utils import run_bass_kernel_spmd

F32 = mybir.dt.float32
BF16 = mybir.dt.bfloat16
AF = mybir.ActivationFunctionType
NP_BF16 = ml_dtypes.bfloat16

B, C, P, D, G = 8, 16, 512, 1024, 512
R = C * P            # 8192 rows per batch
MEGA = 512           # rows per mega-tile
NSUB = MEGA // 128   # 4
NMEGA = R // MEGA    # 16
KD = D // 128        # 8 d-chunks
KG = G // 128        # 4 g-chunks
DA = D + 1           # mm2 moving columns: [ones | bw]
# mm2 free-groups (psum bank caps a group at 512 f32 columns)
MM2_GROUPS = [(0, 343), (343, 685), (685, 1025)]
EPS = 1e-6
N_CORES = 8
# dummy matmuls to exit the PE cold p-state; sized to end right when the
# first qzT tile lands (~11.3us) so no stall separates warmup from work
N_WARM = 44


def build_kernel():
    nc = bacc.Bacc("TRN2", target_bir_lowering=False)
    qzT_d = nc.dram_tensor("qzT", [D, R], BF16, kind="ExternalInput")
    bwT_d = nc.dram_tensor("bwT", [D, G], BF16, kind="ExternalInput")
    bwA_d = nc.dram_tensor("bwA", [G, DA], BF16, kind="ExternalInput")
    msg_d = nc.dram_tensor("msg", [R, D], BF16, kind="ExternalOutput")

    with tile.TileContext(nc) as tc, ExitStack() as ctx:
        const_pool = ctx.enter_context(tc.tile_pool(name="const", bufs=1))
        in_pool = ctx.enter_context(tc.tile_pool(name="inp", bufs=2))
        qgr_pool = ctx.enter_context(tc.tile_pool(name="qgrp", bufs=2))
        out_pool = ctx.enter_context(tc.tile_pool(name="outp", bufs=2))
        small_pool = ctx.enter_context(tc.tile_pool(name="smallp", bufs=4))
        qg_psum = ctx.enter_context(tc.tile_pool(name="qgps", bufs=2, space="PSUM"))
        msg_psum = ctx.enter_context(tc.tile_pool(name="msgps", bufs=5, space="PSUM"))
        warm_psum = ctx.enter_context(tc.tile_pool(name="warmps", bufs=1, space="PSUM"))

        # p-state warmup: the PE reaches full clock only after ~3us of
        # continuous execution; burn the DMA-bound head on dummy matmuls.
        scratch = const_pool.tile([128, 128], BF16)
        nc.vector.memset(scratch, 0.0)
        warm_ps = warm_psum.tile([128, 128], F32)
        for _ in range(N_WARM):
            nc.tensor.matmul(warm_ps, scratch, scratch, start=True, stop=True)

        # Weights go on the scalar-engine DGE ring so they don't queue behind
        # the qzT stream; bwT first and in quarters - it gates mm1.
        bwT_sb = const_pool.tile([128, KD, G], BF16)
        bwT_view = bwT_d[:].rearrange("(k p) g -> p k g", p=128)
        for q in range(4):
            nc.scalar.dma_start(
                out=bwT_sb[:, 2 * q:2 * q + 2, :], in_=bwT_view[:, 2 * q:2 * q + 2, :]
            )
        # bw augmented [g, 1+d] -> [128, gc, DA]  (mm2 moving operand)
        bw_sb = const_pool.tile([128, KG, DA], BF16)
        nc.scalar.dma_start(
            out=bw_sb, in_=bwA_d[:].rearrange("(gc p) d -> p gc d", p=128)
        )

        def load_qzT(t):
            # first megas load in quarters (earlier first matmul); steady
            # state uses halves (better DMA efficiency)
            qzT = in_pool.tile([128, KD, MEGA], BF16, name="qzT")
            qzT_view = qzT_d[:, t * MEGA:(t + 1) * MEGA].rearrange(
                "(k p) r -> p k r", p=128
            )
            nq = 4 if t < 2 else 2
            step = KD // nq
            for q in range(nq):
                nc.sync.dma_start(
                    out=qzT[:, step * q:step * (q + 1), :],
                    in_=qzT_view[:, step * q:step * (q + 1), :],
                )
            return qzT

        pend_qzT = [load_qzT(0), load_qzT(1)]
        store_rr = 0

        for t in range(NMEGA):
            qzT = pend_qzT.pop(0)
            if t + 2 < NMEGA:
                pend_qzT.append(load_qzT(t + 2))

            # ---- mm1: qgT[gc] = sum_k bwT[:,k,gc].T @ qzT[:,k,:] -> relu ----
            qgr = qgr_pool.tile([128, KG, MEGA], BF16, name="qgr")
            for gc in range(KG):
                qg_ps = qg_psum.tile([128, MEGA], F32, name="qg_ps")
                for k in range(KD):
                    nc.tensor.matmul(
                        qg_ps,
                        bwT_sb[:, k, gc * 128:(gc + 1) * 128],
                        qzT[:, k, :],
                        start=(k == 0),
                        stop=(k == KD - 1),
                    )
                nc.scalar.activation(qgr[:, gc, :], qg_ps, AF.Relu)

            # ---- mm2: msg[s] = sum_gc qgr[:,gc,s].T @ bwA[gc], scaled by
            # 1/max(psum col 0, eps) which holds sum_g qgr for these rows ----
            msg_sb = out_pool.tile([128, NSUB, D], BF16, name="msg_sb")
            for s in range(NSUB):
                sc_sb = None
                for gi, (c0, c1) in enumerate(MM2_GROUPS):
                    m_ps = msg_psum.tile([128, c1 - c0], F32, name="m_ps")
                    for gc in range(KG):
                        nc.tensor.matmul(
                            m_ps,
                            qgr[:, gc, s * 128:(s + 1) * 128],
                            bw_sb[:, gc, c0:c1],
                            start=(gc == 0),
                            stop=(gc == KG - 1),
                        )
                    if gi == 0:
                        sc_sb = small_pool.tile([128, 1], F32, name="sc_sb")
                        nc.vector.tensor_scalar_max(sc_sb, m_ps[:, 0:1], EPS)
                        nc.vector.reciprocal(sc_sb, sc_sb)
                        nc.vector.tensor_scalar_mul(
                            msg_sb[:, s, 0:342], m_ps[:, 1:343], sc_sb
                        )
                    elif gi == 1:
                        nc.scalar.mul(
                            msg_sb[:, s, c0 - 1:c1 - 1], m_ps, sc_sb
                        )
                    else:
                        nc.vector.tensor_scalar_mul(
                            msg_sb[:, s, c0 - 1:c1 - 1], m_ps, sc_sb
                        )
                # per-sub store on the scalar ring (weights are long done; a
                # ring sustains >200GB/s so 16MB of stores is light duty).
                # The gpsimd ring is never used for DMA: it costs a fixed
                # ~6.3us terminal drain. The last mega alternates with the
                # sync ring (done loading by then) for a wider final drain.
                row0 = t * MEGA + s * 128
                if t == NMEGA - 1:
                    for h in range(2):
                        ring = nc.scalar if store_rr % 2 == 0 else nc.sync
                        ring.dma_start(
                            out=msg_d[row0:row0 + 128, 512 * h:512 * (h + 1)],
                            in_=msg_sb[:, s, 512 * h:512 * (h + 1)],
                        )
                        store_rr += 1
                else:
                    nc.scalar.dma_start(
                        out=msg_d[row0:row0 + 128, :],
                        in_=msg_sb[:, s, :],
                    )

    nc.compile()
    return nc


_NC_CACHE = None


def _get_nc():
    global _NC_CACHE
    if _NC_CACHE is None:
        _NC_CACHE = build_kernel()
    return _NC_CACHE


def kernel(qz: np.ndarray, binary_weight: np.ndarray) -> np.ndarray:
    qz = np.asarray(qz, dtype=np.float32)
    bw = np.asarray(binary_weight, dtype=np.float32)
    assert qz.shape == (B, C, P, D), qz.shape
    assert bw.shape == (B, G, D), bw.shape

    nc = _get_nc()
    ones = np.ones((G, 1), dtype=np.float32)
    in_maps = []
    for i in range(N_CORES):
        qzT = np.ascontiguousarray(qz[i].reshape(R, D).T).astype(NP_BF16)
        bwT = np.ascontiguousarray(bw[i].T).astype(NP_BF16)
        bwA = np.concatenate([ones, bw[i]], axis=1).astype(NP_BF16)
        in_maps.append({"qzT": qzT, "bwT": bwT, "bwA": bwA})
    res = run_bass_kernel_spmd(nc, in_maps, core_ids=list(range(N_CORES)))
    out = np.stack(
        [
            res.results[i]["msg"].astype(np.float32).reshape(C, P, D)
            for i in range(N_CORES)
        ],
        axis=0,
    )
    return out
